# revision 6
# baseline (speedup 1.0000x reference)
"""Trainium2 Bass kernel for 16-head causal MHA (B=4, S=2048, D=1024).

Sharding: 8 cores = 4 batches x 2 head-groups (8 heads each).
Each core computes QKV projections for its (batch, head-group) slice,
causal softmax attention (probs written causally; masked region stays
zero via pre-zeroed output buffers), and a partial output projection.
Host sums the two head-group partials per batch and adds bo.

Device dataflow per core:
  phase 1: QT/KT = (W^T x^T) in [d_head, S] layout, V in [S, d_head]
           layout (bf16, with a ones column per head for row sums).
           Bias support via an augmented contraction row (x^T row 1024
           is ones; W row 1024 is the bias).
  phase 2 per head:
    A-side (attn_prob output): S = Q K^T in [q, k] layout -> diag mask
           add (-1e30) -> ACT exp with accum_out row sums -> DVE
           normalize by reciprocal -> DMA causal rows to HBM.
    B-side (context): S^T = K Q^T in [k, q] layout -> diag mask ->
           exp (bf16) -> PV matmul with ones-augmented V accumulating
           context^T and row sums in PSUM -> normalize via
           gpsimd partition_broadcast + DVE multiply into ctx^T stack.
  phase 3: out^partial[q, :] = ctx^T stacked as lhsT @ Wo slice.
"""

import os
import sys

import numpy as np

for _p in ("/opt/trn_rl_repo", "/root/.axon_site/_ro/trn_rl_repo"):
    if os.path.isdir(_p) and _p not in sys.path:
        sys.path.append(_p)

S = 2048
D = 1024
NH = 16
DH = 64
HC = 8            # heads per core
NCORES = 8
DP = 1152         # padded contraction dim: D + 1 bias row + pad to 9*128
SCALE = 0.125     # 1/sqrt(64), folded into Wq on host

# Experiment knobs (test.py may flip these before calling kernel()).
TRACE = False
PA_FP16 = False   # store/normalize attn probs in fp16, cast on DMA out

_NC_CACHE = {}


def _build_bass(pa_fp16):
    import concourse.bacc as bacc
    import concourse.mybir as mybir
    import concourse.tile as tile
    from concourse.bass import ts
    from contextlib import ExitStack

    f32 = mybir.dt.float32
    f32r = mybir.dt.float32r
    bf16 = mybir.dt.bfloat16
    f16 = mybir.dt.float16
    AF = mybir.ActivationFunctionType
    ALU = mybir.AluOpType
    PA_DT = f16 if pa_fp16 else f32

    nc = bacc.Bacc("TRN2", target_bir_lowering=False, debug=False,
                   num_devices=NCORES)

    xT = nc.dram_tensor("xT", [DP, S], f32r, kind="ExternalInput").ap()
    wq = nc.dram_tensor("wq", [DP, 512], f32r, kind="ExternalInput").ap()
    wk = nc.dram_tensor("wk", [DP, 512], f32r, kind="ExternalInput").ap()
    wv = nc.dram_tensor("wv", [DP, 512], f32r, kind="ExternalInput").ap()
    wo = nc.dram_tensor("wo", [512, D], f32r, kind="ExternalInput").ap()
    mtA_d = nc.dram_tensor("mtA", [128, 128], f32, kind="ExternalInput").ap()
    mtB_d = nc.dram_tensor("mtB", [128, 128], f32, kind="ExternalInput").ap()
    attn = nc.dram_tensor("attn", [HC, S, S], f32, kind="ExternalOutput").ap()
    outp = nc.dram_tensor("outp", [S, D], f32, kind="ExternalOutput").ap()

    def mm(out, lhsT, rhs, start, stop):
        nc.tensor.matmul(out, lhsT, rhs,
                         start=start, stop=stop, skip_group_check=True)

    with tile.TileContext(nc) as tc, ExitStack() as ctx:
        persist = ctx.enter_context(tc.tile_pool(name="persist", bufs=1))
        QT = [persist.tile([128, S], f32r, tag=f"QT{i}", name=f"QT{i}")
              for i in range(4)]
        KT = [persist.tile([128, S], f32r, tag=f"KT{i}", name=f"KT{i}")
              for i in range(4)]
        VT = [persist.tile([128, HC * 65], bf16, tag=f"VT{i}", name=f"VT{i}")
              for i in range(16)]
        mtA = persist.tile([128, 128], f32, tag="mtA")
        mtB = persist.tile([128, 128], f32, tag="mtB")
        nc.sync.dma_start(mtA[:], mtA_d)
        nc.sync.dma_start(mtB[:], mtB_d)

        # ---------------- phase 1: projections ----------------
        with tc.tile_pool(name="xtp", bufs=1) as xtp, \
             tc.tile_pool(name="wp", bufs=9) as wp, \
             tc.tile_pool(name="pp", bufs=4, space="PSUM") as pp:
            xts = []
            for kt in range(9):
                t = xtp.tile([128, S], f32r, tag=f"xt{kt}")
                nc.sync.dma_start(t[:], xT[ts(kt, 128), :])
                xts.append(t)
            for wdram, dest in ((wq, QT), (wk, KT)):
                wtiles = []
                for kt in range(9):
                    t = wp.tile([128, 512], f32r, tag="w")
                    nc.sync.dma_start(t[:], wdram[ts(kt, 128), :])
                    wtiles.append(t)
                for pt in range(4):
                    for ns in range(4):
                        ps = pp.tile([128, 512], f32, tag="pp")
                        for kt in range(9):
                            mm(ps[:], wtiles[kt][:, ts(pt, 128)],
                               xts[kt][:, ts(ns, 512)], kt == 0, kt == 8)
                        nc.vector.tensor_copy(dest[pt][:, ts(ns, 512)], ps[:])
            wtiles = []
            for kt in range(9):
                t = wp.tile([128, 512], f32r, tag="w")
                nc.sync.dma_start(t[:], wv[ts(kt, 128), :])
                wtiles.append(t)
            for st in range(16):
                ps = pp.tile([128, 512], f32, tag="pp")
                for kt in range(9):
                    mm(ps[:], xts[kt][:, ts(st, 128)], wtiles[kt][:],
                       kt == 0, kt == 8)
                vt3 = VT[st][:].rearrange("p (h c) -> p h c", c=65)
                nc.vector.tensor_copy(
                    vt3[:, :, 0:64],
                    ps[:].rearrange("p (h c) -> p h c", c=64))
                nc.gpsimd.memset(vt3[:, :, 64:65], 1.0)

        # ---------------- phase 2: attention ----------------
        ctp = ctx.enter_context(tc.tile_pool(name="ctp", bufs=1))
        CT = [ctp.tile([128, S], f32r, tag=f"CT{i}", name=f"CT{i}")
              for i in range(4)]
        with tc.tile_pool(name="pa", bufs=3) as pa_pool, \
             tc.tile_pool(name="pb", bufs=3) as pb_pool, \
             tc.tile_pool(name="small", bufs=2) as small, \
             tc.tile_pool(name="psA", bufs=2, space="PSUM") as psA, \
             tc.tile_pool(name="psB", bufs=2, space="PSUM") as psB, \
             tc.tile_pool(name="psC", bufs=2, space="PSUM") as psC:
            for h in range(HC):
                hp, ho = h // 2, 64 * (h % 2)
                QTh = QT[hp][ho:ho + 64, :]
                KTh = KT[hp][ho:ho + 64, :]
                sums = small.tile([128, 16], f32, tag="sums")
                recA = small.tile([128, 16], f32, tag="recA")
                # ---- A-side: normalized probs in [q, k] layout ----
                for qt in range(16):
                    L = 128 * (qt + 1)
                    nchunk = (L + 1023) // 1024
                    acc = small.tile([128, 2], f32, tag="acc")
                    pats = []
                    for ch in range(nchunk):
                        c0 = 1024 * ch
                        cw = min(1024, L - c0)
                        SA = psA.tile([128, 1024], f32, tag="SA")
                        for ns0 in range(0, cw, 512):
                            w = min(512, cw - ns0)
                            mm(SA[:, ns0:ns0 + w], QTh[:, ts(qt, 128)],
                               KTh[:, c0 + ns0:c0 + ns0 + w], True, True)
                        dc = qt * 128 - c0  # diag block is in last chunk
                        if 0 <= dc < cw:
                            nc.vector.tensor_tensor(
                                SA[:, dc:dc + 128], SA[:, dc:dc + 128],
                                mtA[:], op=ALU.add)
                        PAt = pa_pool.tile([128, 1024], PA_DT, tag="pa")
                        nc.scalar.activation(
                            PAt[:, :cw], SA[:, :cw], AF.Exp,
                            accum_out=acc[:, ch:ch + 1])
                        pats.append((PAt, c0, cw))
                    if nchunk == 2:
                        nc.vector.tensor_tensor(
                            sums[:, qt:qt + 1], acc[:, 0:1], acc[:, 1:2],
                            op=ALU.add)
                    else:
                        nc.vector.tensor_copy(sums[:, qt:qt + 1], acc[:, 0:1])
                    nc.vector.reciprocal(recA[:, qt:qt + 1],
                                         sums[:, qt:qt + 1])
                    for PAt, c0, cw in pats:
                        nc.vector.tensor_scalar_mul(
                            PAt[:, :cw], PAt[:, :cw], recA[:, qt:qt + 1])
                        if pa_fp16:
                            nc.gpsimd.dma_start(
                                attn[h, ts(qt, 128), c0:c0 + cw],
                                PAt[:, :cw])
                        else:
                            nc.sync.dma_start(
                                attn[h, ts(qt, 128), c0:c0 + cw],
                                PAt[:, :cw])
                # ---- B-side: context via [k, q] layout ----
                for qs in range(4):
                    CTX = psC.tile([65, 512], f32, tag="ctx")
                    nkt = 4 * qs + 4
                    for kt in range(nkt):
                        o = kt * 128 - qs * 512
                        lo = max(o, 0)
                        SB = psB.tile([128, 512], f32, tag="SB")
                        mm(SB[:, lo:512], KTh[:, ts(kt, 128)],
                           QTh[:, qs * 512 + lo:qs * 512 + 512], True, True)
                        if o >= 0:
                            nc.vector.tensor_tensor(
                                SB[:, o:o + 128], SB[:, o:o + 128], mtB[:],
                                op=ALU.add)
                        PBt = pb_pool.tile([128, 512], bf16, tag="pb")
                        nc.scalar.activation(PBt[:, lo:512], SB[:, lo:512],
                                             AF.Exp)
                        nc.tensor.matmul(
                            CTX[:, lo:512],
                            VT[kt][:, h * 65:(h + 1) * 65],
                            PBt[:, lo:512],
                            start=(kt == 0), stop=(kt == nkt - 1),
                            skip_group_check=True)
                    recB = small.tile([1, 512], f32, tag="recB")
                    nc.vector.reciprocal(recB[:], CTX[64:65, 0:512])
                    recBb = small.tile([64, 512], f32, tag="recBb")
                    nc.gpsimd.partition_broadcast(recBb[:], recB[:])
                    nc.vector.tensor_tensor(
                        CT[hp][ho:ho + 64, ts(qs, 512)], CTX[0:64, 0:512],
                        recBb[:], op=ALU.mult)

        # ---------------- phase 3: output projection ----------------
        if True:
            with tc.tile_pool(name="wop", bufs=1) as wop, \
                 tc.tile_pool(name="osb", bufs=3) as osb, \
                 tc.tile_pool(name="psO", bufs=2, space="PSUM") as psO:
                wot = []
                for pt in range(4):
                    t = wop.tile([128, D], f32r, tag=f"wo{pt}")
                    nc.sync.dma_start(t[:], wo[ts(pt, 128), :])
                    wot.append(t)
                for qt in range(16):
                    OP = psO.tile([128, D], f32, tag="OP")
                    for nb in range(2):
                        for pt in range(4):
                            mm(OP[:, ts(nb, 512)], CT[pt][:, ts(qt, 128)],
                               wot[pt][:, ts(nb, 512)], pt == 0, pt == 3)
                    OS = osb.tile([128, D], f32, tag="os")
                    nc.vector.tensor_copy(OS[:], OP[:])
                    nc.sync.dma_start(outp[ts(qt, 128), :], OS[:])

    nc.compile()
    return nc


def _get_nc():
    key = bool(PA_FP16)
    if key not in _NC_CACHE:
        _NC_CACHE[key] = _build_bass(key)
    return _NC_CACHE[key]


def _build_masks():
    i = np.arange(128)
    # A layout [q part, k free]: mask k > q within diag block
    mtA = np.where(i[None, :] > i[:, None], np.float32(-1e30),
                   np.float32(0.0)).astype(np.float32)
    # B layout [k part, q free]: mask q < k within diag block
    mtB = np.where(i[None, :] < i[:, None], np.float32(-1e30),
                   np.float32(0.0)).astype(np.float32)
    return mtA, mtB


def kernel(x, attn_mask, Wq, bq, Wk, bk, Wv, bv, Wo, bo):
    from concourse.bass_utils import run_bass_kernel_spmd

    x = np.asarray(x, dtype=np.float32)
    Wq = np.asarray(Wq, dtype=np.float32)
    Wk = np.asarray(Wk, dtype=np.float32)
    Wv = np.asarray(Wv, dtype=np.float32)
    Wo = np.asarray(Wo, dtype=np.float32)
    bq = np.asarray(bq, dtype=np.float32)
    bk = np.asarray(bk, dtype=np.float32)
    bv = np.asarray(bv, dtype=np.float32)
    bo = np.asarray(bo, dtype=np.float32)

    mtA, mtB = _build_masks()
    nc = _get_nc()

    in_maps = []
    for c in range(NCORES):
        b, g = c // 2, c % 2
        cs = slice(512 * g, 512 * (g + 1))
        xTa = np.zeros((DP, S), np.float32)
        xTa[:D] = x[b].T
        xTa[D] = 1.0
        wqc = np.zeros((DP, 512), np.float32)
        wqc[:D] = Wq[:, cs] * SCALE
        wqc[D] = bq[cs] * SCALE
        wkc = np.zeros((DP, 512), np.float32)
        wkc[:D] = Wk[:, cs]
        wkc[D] = bk[cs]
        wvc = np.zeros((DP, 512), np.float32)
        wvc[:D] = Wv[:, cs]
        wvc[D] = bv[cs]
        woc = np.ascontiguousarray(Wo[cs, :])
        in_maps.append({"xT": xTa, "wq": wqc, "wk": wkc, "wv": wvc,
                        "wo": woc, "mtA": mtA, "mtB": mtB})

    res = run_bass_kernel_spmd(nc, in_maps, core_ids=list(range(NCORES)),
                               trace=TRACE)
    kernel.last_result = res

    attn_prob = np.empty((4, NH, S, S), np.float32)
    output = np.empty((4, S, D), np.float32)
    for c in range(NCORES):
        b, g = c // 2, c % 2
        attn_prob[b, HC * g:HC * (g + 1)] = res.results[c]["attn"]
    for b in range(4):
        output[b] = res.results[2 * b]["outp"] + res.results[2 * b + 1]["outp"]
        output[b] += bo
    return output, attn_prob
